# revision 41
# baseline (speedup 1.0000x reference)
"""Trainium2 Bass kernel for nn_FeatureRefinement.

Reference computation (bs=16, vl=1024, ql=64, d=1024):
    corr = einsum('bqd,bvd->bqv', Q, V); scores = softmax(corr, axis=1)
    corr_matrix = einsum('bqv,qd->bvd', scores, cor_w)     # cor_w constant over q
    sentence    = WeightedPool(Q)                           # (bs, d)
    sim         = cosine(V, sentence) + log(video_mask)     # (bs, vl)
    features    = concat([V, sim*sim_w, sentence_bcast, corr_matrix], -1)
    out         = relu(features @ mixer_w + mixer_b)

Algebraic restructuring (exact up to fp rounding):
  - softmax over q sums to 1  =>  corr_matrix[b,v,:] == cor_v_w*cor_q_w  (constant)
  - sim_features @ W2  == sim[b,v] * (sim_w.T @ W2)        (rank-1)
  - pooled_query @ W3  == sentence[b] @ W3                 (rank-1 per batch)
  so   out = relu(V @ W1 + [sim; 1; 1]^T @ [w2v; bias_hi; bias_lo])
  The only heavy compute is V @ W1 (4x FLOP reduction).

Kernel strategy (v4):
  - V arrives pre-transposed AND pre-swizzled from the host so every DMA
    is a linear per-partition copy; the PE never transposes V.
  - The main V@W1 MMs run at the fp16 roofline (216ns/MM, measured).
  - All small matmuls (aug rank-3, bias, dot rows) are packed two-at-a-
    time into disjoint 32-row/col groups of the PE array (tile_position
    concurrency) so they cost ~half their serial time.
  - Row norms come from an fp8 natural-layout copy squared on the scalar
    engine; one bf16 PE transpose per 512-row slab.
  - DMA priority: w1 + first V slab + qt lead all three issue queues so
    the heavy MMs start as soon as the shared HBM bandwidth allows.

Sharding: data-parallel over batch, 2 batches per core on 8 cores. No
collectives; host scatters inputs / gathers outputs.
"""
import sys

sys.path.insert(0, "/opt/trn_rl_repo")

import numpy as np
import ml_dtypes
from contextlib import ExitStack

import concourse.bass as bass
import concourse.tile as tile
from concourse import bacc, mybir
from concourse.bass_utils import run_bass_kernel_spmd
from concourse.masks import make_identity


def _install_ntff_shim():
    """This container's antenv lacks axon_hooks; if tracing is requested
    (BASS_TRACE=1), run_bass_kernel_spmd would crash importing it. Provide
    the hook via trn_agent_boot's ctypes helper, and keep the trace
    post-processing local (no bucket uploads)."""
    import types, os
    try:
        import antenv  # noqa: F401
        import antenv.axon_hooks  # noqa: F401
        return  # already present
    except ImportError:
        pass
    try:
        import trn_agent_boot.trn_boot as _tb
        hook = _tb._ntff_profile_via_ctypes("/opt/axon/libaxon_pjrt.so")
        mod = types.ModuleType("antenv.axon_hooks")
        mod.get_axon_ntff_profile_hook = lambda: hook
        sys.modules["antenv.axon_hooks"] = mod
        from concourse import bass_utils as _bu
        _orig = _bu.upload_artifacts

        def _safe_upload(tmpdir):
            try:
                return _orig(tmpdir)
            except Exception:
                return f"file://{tmpdir}"

        _bu.upload_artifacts = _safe_upload
    except Exception:
        pass


_install_ntff_shim()

F32 = mybir.dt.float32
F16 = mybir.dt.float16
BF16 = mybir.dt.bfloat16
F8 = mybir.dt.float8e4
AF = mybir.ActivationFunctionType
AX = mybir.AxisListType
ALU = mybir.AluOpType

BS, VL, QL, D = 16, 1024, 64, 1024
NCORES = 8
BPC = BS // NCORES          # batches per core
KC = D // 128               # contraction chunks
SS = 512                    # v-rows per super-slab
NSS = VL // SS              # slabs per batch (2)
NSLAB = BPC * NSS           # slabs per core (4)
NEG_INF = -1e30

VDT = F16                   # dtype of the heavy V @ W1 path


def _build_program():
    nc = bacc.Bacc("TRN2", target_bir_lowering=False, debug=False, num_devices=NCORES)

    # vt/v8/w1/w3/qt arrive pre-swizzled so each DMA is a linear
    # per-partition copy (small packets gutted DMA throughput otherwise)
    vt_d = nc.dram_tensor("vt", [NSLAB, 128, KC * SS], VDT, kind="ExternalInput").ap()
    v8_d = nc.dram_tensor("v8", [NSLAB, 128, 4 * D], F8, kind="ExternalInput").ap()
    qt_d = nc.dram_tensor("qt", [128, KC * 2 * QL + KC], F16, kind="ExternalInput").ap()
    q2_d = nc.dram_tensor("q2", [2 * QL, D], F16, kind="ExternalInput").ap()
    qb_d = nc.dram_tensor("qb", [1, 2 * QL], F32, kind="ExternalInput").ap()
    vb_d = nc.dram_tensor("vb", [BPC, VL], F32, kind="ExternalInput").ap()
    w1_d = nc.dram_tensor("w1", [128, KC * D], VDT, kind="ExternalInput").ap()
    w3_d = nc.dram_tensor("w3", [128, KC * D], VDT, kind="ExternalInput").ap()
    w2v_d = nc.dram_tensor("w2v", [1, D], BF16, kind="ExternalInput").ap()
    biasc_d = nc.dram_tensor("biasc", [2, D], F32, kind="ExternalInput").ap()
    aones_d = nc.dram_tensor("aones", [4, NSLAB * SS], BF16, kind="ExternalInput").ap()
    out_d = nc.dram_tensor("out", [BPC, VL, D], F32, kind="ExternalOutput").ap()

    with tile.TileContext(nc) as tc, ExitStack() as ctx:
        singles = ctx.enter_context(tc.tile_pool(name="singles", bufs=1))
        qstuff = ctx.enter_context(tc.tile_pool(name="qstuff", bufs=1))
        rows = ctx.enter_context(tc.tile_pool(name="rows", bufs=2))
        vtpool = ctx.enter_context(tc.tile_pool(name="vtpool", bufs=4))
        v8pool = ctx.enter_context(tc.tile_pool(name="v8pool", bufs=4))
        trashp = ctx.enter_context(tc.tile_pool(name="trashp", bufs=2))
        opool = ctx.enter_context(tc.tile_pool(name="opool", bufs=3))
        decpool = ctx.enter_context(tc.tile_pool(name="decpool", bufs=4))
        psMain = ctx.enter_context(tc.tile_pool(name="psMain", bufs=6, space="PSUM"))
        psDot = ctx.enter_context(tc.tile_pool(name="psDot", bufs=1, space="PSUM"))
        psRow = ctx.enter_context(tc.tile_pool(name="psRow", bufs=1, space="PSUM"))

        # ============== DMA issue (per queue, in priority order) =============
        # Gating set for the first mains: w1 (split on gpsimd+scalar) and
        # vt slab 0 (sync head).  Everything else queues behind.
        vt_tiles, v8_tiles = [], []
        for slab in range(NSLAB):
            vt_tiles.append(
                vtpool.tile([128, KC, SS], VDT, tag="vt", name=f"vt_{slab}"))
            v8_tiles.append(
                v8pool.tile([128, 4, D], F8, tag="v8", name=f"v8_{slab}"))

        def load_vt(eng, slab):
            eng.dma_start(out=vt_tiles[slab],
                          in_=vt_d[slab].rearrange("p (k c) -> p k c", k=KC))

        def load_v8(eng, slab):
            eng.dma_start(out=v8_tiles[slab],
                          in_=v8_d[slab].rearrange("p (s d) -> p s d", s=4))

        load_vt(nc.sync, 0)

        w1_sb = singles.tile([128, KC, D], VDT)
        nc.gpsimd.dma_start(
            out=w1_sb[:, 0 : KC // 2, :],
            in_=w1_d[:, 0 : KC * D // 2].rearrange("p (k d) -> p k d", k=KC // 2))
        nc.gpsimd.dma_start(
            out=w1_sb[:, KC // 2 :, :],
            in_=w1_d[:, KC * D // 2 :].rearrange("p (k d) -> p k d", k=KC // 2))

        # gpsimd compute bits needed early (before the w3 issue)
        ident = singles.tile([128, 128], F32)
        make_identity(nc, ident)
        identB = singles.tile([128, 128], BF16)
        nc.vector.tensor_copy(identB, ident)
        bd_sb = qstuff.tile([2 * QL, BPC], F16)
        nc.gpsimd.memset(bd_sb, 0.0)

        w3_sb = singles.tile([128, KC, D], VDT)
        nc.gpsimd.dma_start(
            out=w3_sb, in_=w3_d.rearrange("p (k d) -> p k d", k=KC))

        # scalar queue: qt+pw packed, then q2 (phase A), then w1 odd chunks
        qtpw_sb = qstuff.tile([128, KC * 2 * QL + KC], F16)
        nc.scalar.dma_start(out=qtpw_sb, in_=qt_d)
        qt_sb = qtpw_sb[:, 0 : KC * 2 * QL].rearrange("p (k q) -> p k q", k=KC)
        pw_sb = qtpw_sb[:, KC * 2 * QL :]
        q2_sb = qstuff.tile([2 * QL, D], F16)
        nc.scalar.dma_start(out=q2_sb, in_=q2_d)
        # small mid-kernel tensors on the SCALAR queue: tiny transfers don't
        # trip the one-outstanding-transfer issue backpressure that was
        # holding them behind vt slabs until ~26-32us on the sync queue
        qb_sb = qstuff.tile([1, 2 * QL], F32)
        nc.scalar.dma_start(out=qb_sb, in_=qb_d)
        biasc_sb = singles.tile([2, D], F32)
        nc.scalar.dma_start(out=biasc_sb, in_=biasc_d)
        vb_sb = qstuff.tile([1, BPC, VL], F32)
        for b in range(BPC):
            nc.scalar.dma_start(out=vb_sb[:, b, :], in_=vb_d[b : b + 1, :])
        # aug lhsT rows: [sim; 1; 1] per slab, replicated at partition
        # groups 0-2 and 32-34 so two aug MMs can run packed
        aug_all = qstuff.tile([128, NSLAB, SS], BF16)
        nc.scalar.dma_start(out=aug_all[1:3, :, :],
                            in_=aones_d[0:2].rearrange("p (s c) -> p s c", s=NSLAB))
        nc.scalar.dma_start(out=aug_all[33:35, :, :],
                            in_=aones_d[2:4].rearrange("p (s c) -> p s c", s=NSLAB))
        # aug rhs: [w2v; bias_hi; bias_lo] per batch, replicated at 0-2/32-34
        aug_rhs = qstuff.tile([128, BPC, D], BF16)
        for b in range(BPC):
            nc.scalar.dma_start(out=aug_rhs[0:1, b, :], in_=w2v_d)
            nc.scalar.dma_start(out=aug_rhs[32:33, b, :], in_=w2v_d)
        load_v8(nc.scalar, 0)
        load_v8(nc.scalar, 1)
        # sync queue carries only the big slabs
        load_vt(nc.sync, 1)
        load_vt(nc.sync, 2)
        load_v8(nc.sync, 2)
        load_vt(nc.sync, 3)
        load_v8(nc.sync, 3)

        # ============ Phase A: query side (both batches packed) ==============
        al_ps = psRow.tile([1, 2 * QL], F32, tag="psrow")
        for k in range(KC):
            nc.tensor.matmul(al_ps, pw_sb[:, k : k + 1], qt_sb[:, k, :],
                             start=(k == 0), stop=(k == KC - 1))
        alpha_sb = rows.tile([1, 2 * QL], F32)
        nc.vector.tensor_add(alpha_sb, al_ps, qb_sb)

        alphas_sb = rows.tile([1, 2 * QL], F32, tag="alphas", bufs=1)
        for b in range(BPC):
            seg = slice(b * QL, (b + 1) * QL)
            mx = rows.tile([1, 1], F32)
            nc.vector.reduce_max(mx, alpha_sb[:, seg], axis=AX.X)
            asub = rows.tile([1, QL], F32)
            nc.vector.tensor_scalar_sub(asub, alpha_sb[:, seg], mx)
            aexp = rows.tile([1, QL], F32)
            asum = rows.tile([1, 1], F32)
            nc.scalar.activation(aexp, asub, AF.Exp, accum_out=asum)
            rsum = rows.tile([1, 1], F32)
            nc.vector.reciprocal(rsum, asum)
            nc.vector.tensor_scalar_mul(alphas_sb[:, seg], aexp, rsum)

        sentT = qstuff.tile([128, KC, BPC], F16)
        snsq2 = qstuff.tile([1, BPC], F32)

        def phase_a_rest():
            # alphas^T as block-diagonal [128, 2] fp16 (col b = batch b)
            alT_ps = psRow.tile([2 * QL, 1], F32, tag="psrow")
            nc.tensor.transpose(alT_ps, alphas_sb, ident[:1, :1])
            for b in range(BPC):
                seg = slice(b * QL, (b + 1) * QL)
                nc.vector.tensor_copy(bd_sb[seg, b : b + 1], alT_ps[seg, :])
            # sentence^T chunks: sT[dchunk, b] = Q2^T(chunk) @ bd -> [128,2]/k
            sT_ps = psRow.tile([128, KC * BPC], F32, tag="psrow")
            for k in range(KC):
                nc.tensor.matmul(sT_ps[:, k * BPC : (k + 1) * BPC],
                                 q2_sb[:, k * 128 : (k + 1) * 128], bd_sb,
                                 start=True, stop=True)
            nc.vector.tensor_copy(
                sentT, sT_ps.rearrange("p (k b) -> p k b", k=KC))

            # ||sentence||^2 via self-dot matmuls -> [1, 2], clamped
            sn_ps = psRow.tile([1, BPC], F32, tag="psrow")
            for b in range(BPC):
                for k in range(KC):
                    nc.tensor.matmul(sn_ps[:, b : b + 1],
                                     sentT[:, k, b : b + 1],
                                     sentT[:, k, b : b + 1],
                                     start=(k == 0), stop=(k == KC - 1))
            nc.vector.tensor_scalar_max(snsq2, sn_ps, 1e-16)

        # ================= Phase C: video side (heavy) =======================
        pend_aug = []  # (o_ps pair, slab, b, i) awaiting aug+relu+store

        def flush_aug_pair(entries):
            for h in range(2):
                for j, (o_ps2, slab_j, b_j, i_j) in enumerate(entries):
                    nc.tensor.matmul(
                        o_ps2[h],
                        aug_all[32 * j : 32 * j + 3, slab_j,
                                i_j * 128 : (i_j + 1) * 128],
                        aug_rhs[32 * j : 32 * j + 3, b_j,
                                h * 512 : (h + 1) * 512],
                        start=False, stop=True)
            for o_ps2, slab_j, b_j, i_j in entries:
                out_sb = opool.tile([128, D], F32)
                for h in range(2):
                    nc.vector.tensor_scalar_max(
                        out_sb[:, h * 512 : (h + 1) * 512], o_ps2[h], 0.0)
                r0 = (slab_j % NSS) * SS + i_j * 128
                nc.gpsimd.dma_start(out=out_d[b_j, r0 : r0 + 128, :], in_=out_sb)

        def dot_and_sim(slab):
            # dot row + sim chain for one slab (all at partition 0)
            b, ss = divmod(slab, NSS)
            v8_sb = v8_tiles[slab]
            vnsq = rows.tile([128, 4], F32, tag="vnsqc")
            for s4 in range(4):
                vtrash = trashp.tile([128, D], BF16, tag="vtrash")
                nc.scalar.activation(vtrash, v8_sb[:, s4, :], AF.Square,
                                     accum_out=vnsq[:, s4 : s4 + 1])
            vnsq_bf = rows.tile([128, 4], BF16, tag="vnsqb")
            nc.vector.tensor_copy(vnsq_bf, vnsq)

            dps = psDot.tile([1, SS], F32, tag="dot")
            for k in range(KC):
                nc.tensor.matmul(dps, sentT[:, k, b : b + 1],
                                 vt_tiles[slab][:, k, :],
                                 start=(k == 0), stop=(k == KC - 1))
            # norm row: one [128,4]->[4,128] bf16 transpose, then a small
            # DMA gathers the psum rows into a [1,512] sbuf row
            vnr_ps = psRow.tile([4, 128], BF16, tag="psrow")
            nc.tensor.transpose(vnr_ps, vnsq_bf, identB)
            vnr_cp = rows.tile([4, 128], BF16, tag="vnrcp")
            nc.vector.tensor_copy(vnr_cp, vnr_ps)
            vnr_row = rows.tile([1, SS], BF16, tag="vnrrow")
            nc.gpsimd.dma_start(out=vnr_row, in_=vnr_cp)

            # sim = dot * rsqrt(max(vnsq,eps)*snsq) + log(video_mask)
            t1 = rows.tile([1, SS], F32, tag="t1")
            nc.vector.tensor_scalar(t1, vnr_row, 1e-16, snsq2[:, b : b + 1],
                                    op0=ALU.max, op1=ALU.mult)
            t3 = rows.tile([1, SS], F32, tag="t3")
            nc.scalar.activation(t3, t1, AF.Abs_reciprocal_sqrt)
            t4 = rows.tile([1, SS], F32, tag="t4")
            nc.vector.tensor_mul(t4, dps, t3)
            sim_row = rows.tile([1, SS], BF16, tag="simrow")
            nc.vector.tensor_add(sim_row, t4,
                                 vb_sb[:, b, ss * SS : (ss + 1) * SS])
            nc.gpsimd.dma_start(out=aug_all[0:1, slab, :], in_=sim_row)
            nc.gpsimd.dma_start(out=aug_all[32:33, slab, :], in_=sim_row)

        def bias_rows():
            # bias row [2, D] = sentence @ W3 + biasc; h halves col-packed
            b_ps = psRow.tile([128, 512], F32, tag="psrow")
            for k in range(KC):
                nc.tensor.matmul(b_ps[0:2, :], sentT[:, k, :],
                                 w3_sb[:, k, 0:512],
                                 start=(k == 0), stop=(k == KC - 1))
                nc.tensor.matmul(b_ps[32:34, :], sentT[:, k, :],
                                 w3_sb[:, k, 512:1024],
                                 start=(k == 0), stop=(k == KC - 1))
            bias_f = rows.tile([2, D], F32, tag="biasf", bufs=1)
            nc.vector.tensor_add(bias_f[:, 0:512], b_ps[0:2, :],
                                 biasc_sb[:, 0:512])
            # partition-32 psum half lands via a bounce + DMA hop
            bh_cp = rows.tile([34, 512], F32, tag="bhcp", bufs=1)
            nc.vector.tensor_copy(bh_cp[32:34, :], b_ps[32:34, :])
            b_hi = rows.tile([2, 512], F32, tag="bhi", bufs=1)
            nc.gpsimd.dma_start(out=b_hi, in_=bh_cp[32:34, :])
            nc.vector.tensor_add(bias_f[:, 512:1024], b_hi, biasc_sb[:, 512:1024])
            bias_hi = rows.tile([2, D], BF16, tag="biashi", bufs=1)
            nc.vector.tensor_copy(bias_hi, bias_f)
            bias_lo = rows.tile([2, D], BF16, tag="biaslo", bufs=1)
            nc.vector.tensor_sub(bias_lo, bias_f, bias_hi)
            for b2 in range(BPC):
                for p in (1, 33):
                    nc.gpsimd.dma_start(out=aug_rhs[p : p + 1, b2, :],
                                        in_=bias_hi[b2 : b2 + 1, :])
                    nc.gpsimd.dma_start(out=aug_rhs[p + 1 : p + 2, b2, :],
                                        in_=bias_lo[b2 : b2 + 1, :])

        # slab 0 is decoupled: its main partials leave PSUM immediately
        # (banks recycle without waiting on the sim/bias chain); the rank-3
        # aug is applied later into fresh banks + DVE add/relu.
        decoup = []

        def late_assemble(entry):
            dec_sb, slab_j, b_j, i_j = entry
            out_sb = opool.tile([128, D], F32)
            for h in range(2):
                a_ps = psMain.tile([128, 512], F32, tag="o_ps",
                                   name=f"late_{i_j}_{h}")
                nc.tensor.matmul(
                    a_ps, aug_all[0:3, slab_j, i_j * 128 : (i_j + 1) * 128],
                    aug_rhs[0:3, b_j, h * 512 : (h + 1) * 512],
                    start=True, stop=True)
                nc.vector.tensor_add(out_sb[:, h * 512 : (h + 1) * 512],
                                     dec_sb[:, h * 512 : (h + 1) * 512], a_ps)
                nc.vector.tensor_scalar_max(
                    out_sb[:, h * 512 : (h + 1) * 512],
                    out_sb[:, h * 512 : (h + 1) * 512], 0.0)
            r0 = (slab_j % NSS) * SS + i_j * 128
            nc.gpsimd.dma_start(out=out_d[b_j, r0 : r0 + 128, :], in_=out_sb)

        bias_done = False
        for slab in range(NSLAB):
            b, ss = divmod(slab, NSS)
            vt_sb = vt_tiles[slab]
            thresh = 2 if slab == NSLAB - 1 else 3
            for i in range(4):
                if slab == 1 and i < 2:
                    late_assemble(decoup.pop(0))
                    late_assemble(decoup.pop(0))
                if len(pend_aug) >= thresh:
                    flush_aug_pair([pend_aug.pop(0), pend_aug.pop(0)])

                # phase-A rest issues BEFORE t1's mains/copies so its DVE ops
                # queue ahead of the decoupling copies (they were delaying the
                # block-diagonal build and stalling the sentence MMs ~3us)
                if slab == 0 and i == 1:
                    phase_a_rest()

                o_ps2 = [psMain.tile([128, 512], F32, tag="o_ps",
                                     name=f"o_ps_{slab}_{i}_{h}")
                         for h in range(2)]
                # 8 consecutive MMs per PSUM bank (bank alternation per
                # instruction triggers the PE depth-cycling penalty)
                for h in range(2):
                    for k in range(KC):
                        nc.tensor.matmul(
                            o_ps2[h], vt_sb[:, k, i * 128 : (i + 1) * 128],
                            w1_sb[:, k, h * 512 : (h + 1) * 512],
                            start=(k == 0),
                            stop=(slab == 0 and k == KC - 1))
                if slab == 0:
                    dec_sb = decpool.tile([128, D], F32, tag="dec",
                                          name=f"dec_{i}")
                    for h in range(2):
                        nc.vector.tensor_copy(
                            dec_sb[:, h * 512 : (h + 1) * 512], o_ps2[h])
                    decoup.append((dec_sb, slab, b, i))
                else:
                    pend_aug.append((o_ps2, slab, b, i))

                if i == 1:
                    dot_and_sim(slab)
                if i == 2 and not bias_done:
                    bias_rows()
                    bias_done = True

        while pend_aug:
            take = pend_aug[:2]
            pend_aug = pend_aug[2:]
            flush_aug_pair(take)

    nc.compile()
    return nc


_NC = None
_LAST_RESULTS = None


def _get_program():
    global _NC
    if _NC is None:
        _NC = _build_program()
    return _NC


def kernel(video_features, query_features, video_mask, query_mask,
           sim_w, cor_v_w, cor_q_w, pool_w, mixer_w, mixer_b):
    video_features = np.asarray(video_features, dtype=np.float32)
    query_features = np.asarray(query_features, dtype=np.float32)
    video_mask = np.asarray(video_mask, dtype=np.float32)
    query_mask = np.asarray(query_mask, dtype=np.float32)
    sim_w = np.asarray(sim_w, dtype=np.float32)
    cor_v_w = np.asarray(cor_v_w, dtype=np.float32)
    cor_q_w = np.asarray(cor_q_w, dtype=np.float32)
    pool_w = np.asarray(pool_w, dtype=np.float32)
    mixer_w = np.asarray(mixer_w, dtype=np.float32)
    mixer_b = np.asarray(mixer_b, dtype=np.float32)

    # host-side folds of the weight-only algebra (O(d^2), negligible)
    W1 = np.ascontiguousarray(mixer_w[0:D]).astype(np.float16)
    W2 = mixer_w[D : 2 * D]
    W3 = np.ascontiguousarray(mixer_w[2 * D : 3 * D]).astype(np.float16)
    W4 = mixer_w[3 * D : 4 * D]
    w2v = (sim_w[:, 0] @ W2.astype(np.float32)).astype(ml_dtypes.bfloat16)[None, :]
    cor_vec = (cor_v_w[0] * cor_q_w[0, 0]).astype(np.float32)
    biasc = np.broadcast_to((cor_vec @ W4 + mixer_b).astype(np.float32)[None, :],
                            (2, D)).copy()
    qbias = ((1.0 - query_mask) * NEG_INF).astype(np.float32)
    vbias = np.log(video_mask + 1e-45).astype(np.float32)
    pw2 = np.ascontiguousarray(pool_w[:, 0].reshape(KC, 128).T).astype(np.float16)
    aones = np.ones((4, NSLAB * SS), dtype=ml_dtypes.bfloat16)

    v16 = video_features.astype(np.float16)
    # vt: [BS*NSS, 128, KC*SS] with vt[slab, p, k*SS+c] = V[b, ss*SS+c, k*128+p]
    vt_full = np.ascontiguousarray(
        v16.transpose(0, 2, 1)                 # [BS, D, VL]
        .reshape(BS, KC, 128, NSS, SS)
        .transpose(0, 3, 2, 1, 4)              # [BS, NSS, 128, KC, SS]
        .reshape(BS * NSS, 128, KC * SS))
    # v8: [BS*NSS, 128, 4*D] with v8[slab, p, s4*D+d] = V[b, ss*SS+s4*128+p, d]
    v8_full = np.ascontiguousarray(
        video_features.astype(ml_dtypes.float8_e4m3fn)
        .reshape(BS, NSS, 4, 128, D)
        .transpose(0, 1, 3, 2, 4)              # [BS, NSS, 128, 4, D]
        .reshape(BS * NSS, 128, 4 * D))
    q16 = query_features.astype(np.float16)
    W1s = np.ascontiguousarray(                # [128, KC*D]
        W1.reshape(KC, 128, D).transpose(1, 0, 2).reshape(128, KC * D))
    W3s = np.ascontiguousarray(
        W3.reshape(KC, 128, D).transpose(1, 0, 2).reshape(128, KC * D))

    nc = _get_program()
    in_maps = []
    for c in range(NCORES):
        sl = slice(c * BPC, (c + 1) * BPC)
        qc = q16[sl]                                             # [2, QL, D]
        qt = np.ascontiguousarray(np.concatenate([  # [128, KC*2QL + KC]
            qc.transpose(2, 0, 1).reshape(KC, 128, 2 * QL)
            .transpose(1, 0, 2).reshape(128, KC * 2 * QL),
            pw2], axis=1))
        q2 = np.ascontiguousarray(qc.reshape(2 * QL, D))
        slsl = slice(c * NSLAB, (c + 1) * NSLAB)
        in_maps.append({
            "vt": vt_full[slsl],
            "v8": v8_full[slsl],
            "qt": qt,
            "q2": q2,
            "qb": np.ascontiguousarray(qbias[sl].reshape(1, 2 * QL)),
            "vb": np.ascontiguousarray(vbias[sl]),
            "w1": W1s,
            "w3": W3s,
            "w2v": w2v,
            "biasc": biasc,
            "aones": aones,
        })
    res = run_bass_kernel_spmd(nc, in_maps, core_ids=list(range(NCORES)))
    global _LAST_RESULTS
    _LAST_RESULTS = res
    out = np.concatenate([res.results[c]["out"] for c in range(NCORES)], axis=0)
    return out.astype(np.float32, copy=False)


# revision 43
# speedup vs baseline: 1.0853x; 1.0853x over previous
"""Trainium2 Bass kernel for nn_FeatureRefinement.

Reference computation (bs=16, vl=1024, ql=64, d=1024):
    corr = einsum('bqd,bvd->bqv', Q, V); scores = softmax(corr, axis=1)
    corr_matrix = einsum('bqv,qd->bvd', scores, cor_w)     # cor_w constant over q
    sentence    = WeightedPool(Q)                           # (bs, d)
    sim         = cosine(V, sentence) + log(video_mask)     # (bs, vl)
    features    = concat([V, sim*sim_w, sentence_bcast, corr_matrix], -1)
    out         = relu(features @ mixer_w + mixer_b)

Algebraic restructuring (exact up to fp rounding):
  - softmax over q sums to 1  =>  corr_matrix[b,v,:] == cor_v_w*cor_q_w  (constant)
  - sim_features @ W2  == sim[b,v] * (sim_w.T @ W2)        (rank-1)
  - pooled_query @ W3  == sentence[b] @ W3                 (rank-1 per batch)
  so   out = relu(V @ W1 + [sim; 1; 1]^T @ [w2v; bias_hi; bias_lo])
  The only heavy compute is V @ W1 (4x FLOP reduction).

Kernel strategy (v4):
  - V arrives pre-transposed AND pre-swizzled from the host so every DMA
    is a linear per-partition copy; the PE never transposes V.
  - The main V@W1 MMs run at the fp16 roofline (216ns/MM, measured).
  - All small matmuls (aug rank-3, bias, dot rows) are packed two-at-a-
    time into disjoint 32-row/col groups of the PE array (tile_position
    concurrency) so they cost ~half their serial time.
  - Row norms come from an fp8 natural-layout copy squared on the scalar
    engine; one bf16 PE transpose per 512-row slab.
  - DMA priority: w1 + first V slab + qt lead all three issue queues so
    the heavy MMs start as soon as the shared HBM bandwidth allows.

Sharding: data-parallel over batch, 2 batches per core on 8 cores. No
collectives; host scatters inputs / gathers outputs.
"""
import sys

sys.path.insert(0, "/opt/trn_rl_repo")

import numpy as np
import ml_dtypes
from contextlib import ExitStack

import concourse.bass as bass
import concourse.tile as tile
from concourse import bacc, mybir
from concourse.bass_utils import run_bass_kernel_spmd
from concourse.masks import make_identity


def _install_ntff_shim():
    """This container's antenv lacks axon_hooks; if tracing is requested
    (BASS_TRACE=1), run_bass_kernel_spmd would crash importing it. Provide
    the hook via trn_agent_boot's ctypes helper, and keep the trace
    post-processing local (no bucket uploads)."""
    import types, os
    try:
        import antenv  # noqa: F401
        import antenv.axon_hooks  # noqa: F401
        return  # already present
    except ImportError:
        pass
    try:
        import trn_agent_boot.trn_boot as _tb
        hook = _tb._ntff_profile_via_ctypes("/opt/axon/libaxon_pjrt.so")
        mod = types.ModuleType("antenv.axon_hooks")
        mod.get_axon_ntff_profile_hook = lambda: hook
        sys.modules["antenv.axon_hooks"] = mod
        from concourse import bass_utils as _bu
        _orig = _bu.upload_artifacts

        def _safe_upload(tmpdir):
            try:
                return _orig(tmpdir)
            except Exception:
                return f"file://{tmpdir}"

        _bu.upload_artifacts = _safe_upload
    except Exception:
        pass


_install_ntff_shim()

F32 = mybir.dt.float32
F16 = mybir.dt.float16
BF16 = mybir.dt.bfloat16
F8 = mybir.dt.float8e4
AF = mybir.ActivationFunctionType
AX = mybir.AxisListType
ALU = mybir.AluOpType

BS, VL, QL, D = 16, 1024, 64, 1024
NCORES = 8
BPC = BS // NCORES          # batches per core
KC = D // 128               # contraction chunks
SS = 512                    # v-rows per super-slab
NSS = VL // SS              # slabs per batch (2)
NSLAB = BPC * NSS           # slabs per core (4)
NEG_INF = -1e30

VDT = F16                   # dtype of the heavy V @ W1 path


def _build_program():
    nc = bacc.Bacc("TRN2", target_bir_lowering=False, debug=False, num_devices=NCORES)

    # vt/v8/w1/w3/qt arrive pre-swizzled so each DMA is a linear
    # per-partition copy (small packets gutted DMA throughput otherwise)
    vt_d = nc.dram_tensor("vt", [NSLAB, 128, KC * SS], VDT, kind="ExternalInput").ap()
    v8_d = nc.dram_tensor("v8", [NSLAB, 128, 4 * D], F8, kind="ExternalInput").ap()
    qt_d = nc.dram_tensor("qt", [128, KC * 2 * QL + KC], F16, kind="ExternalInput").ap()
    q2_d = nc.dram_tensor("q2", [2 * QL, D], F16, kind="ExternalInput").ap()
    qb_d = nc.dram_tensor("qb", [1, 2 * QL], F32, kind="ExternalInput").ap()
    vb_d = nc.dram_tensor("vb", [BPC, VL], F32, kind="ExternalInput").ap()
    w1_d = nc.dram_tensor("w1", [128, KC * D], VDT, kind="ExternalInput").ap()
    w3_d = nc.dram_tensor("w3", [128, KC * D], VDT, kind="ExternalInput").ap()
    w2v_d = nc.dram_tensor("w2v", [1, D], BF16, kind="ExternalInput").ap()
    biasc_d = nc.dram_tensor("biasc", [2, D], F32, kind="ExternalInput").ap()
    aones_d = nc.dram_tensor("aones", [4, NSLAB * SS], BF16, kind="ExternalInput").ap()
    out_d = nc.dram_tensor("out", [BPC, VL, D], F32, kind="ExternalOutput").ap()

    with tile.TileContext(nc) as tc, ExitStack() as ctx:
        singles = ctx.enter_context(tc.tile_pool(name="singles", bufs=1))
        qstuff = ctx.enter_context(tc.tile_pool(name="qstuff", bufs=1))
        rows = ctx.enter_context(tc.tile_pool(name="rows", bufs=2))
        vtpool = ctx.enter_context(tc.tile_pool(name="vtpool", bufs=4))
        v8pool = ctx.enter_context(tc.tile_pool(name="v8pool", bufs=4))
        trashp = ctx.enter_context(tc.tile_pool(name="trashp", bufs=2))
        opool = ctx.enter_context(tc.tile_pool(name="opool", bufs=3))
        decpool = ctx.enter_context(tc.tile_pool(name="decpool", bufs=4))
        psMain = ctx.enter_context(tc.tile_pool(name="psMain", bufs=6, space="PSUM"))
        psDot = ctx.enter_context(tc.tile_pool(name="psDot", bufs=1, space="PSUM"))
        psRow = ctx.enter_context(tc.tile_pool(name="psRow", bufs=1, space="PSUM"))

        # ============== DMA issue (per queue, in priority order) =============
        # Gating set for the first mains: w1 (split on gpsimd+scalar) and
        # vt slab 0 (sync head).  Everything else queues behind.
        vt_tiles, v8_tiles = [], []
        for slab in range(NSLAB):
            vt_tiles.append(
                vtpool.tile([128, KC, SS], VDT, tag="vt", name=f"vt_{slab}"))
            v8_tiles.append(
                v8pool.tile([128, 4, D], F8, tag="v8", name=f"v8_{slab}"))

        def load_vt(eng, slab):
            eng.dma_start(out=vt_tiles[slab],
                          in_=vt_d[slab].rearrange("p (k c) -> p k c", k=KC))

        def load_v8(eng, slab):
            eng.dma_start(out=v8_tiles[slab],
                          in_=v8_d[slab].rearrange("p (s d) -> p s d", s=4))

        load_vt(nc.sync, 0)

        w1_sb = singles.tile([128, KC, D], VDT)
        nc.gpsimd.dma_start(
            out=w1_sb[:, 0 : KC // 2, :],
            in_=w1_d[:, 0 : KC * D // 2].rearrange("p (k d) -> p k d", k=KC // 2))
        nc.gpsimd.dma_start(
            out=w1_sb[:, KC // 2 :, :],
            in_=w1_d[:, KC * D // 2 :].rearrange("p (k d) -> p k d", k=KC // 2))

        # gpsimd compute bits needed early (before the w3 issue)
        ident = singles.tile([128, 128], F32)
        make_identity(nc, ident)
        identB = singles.tile([128, 128], BF16)
        nc.vector.tensor_copy(identB, ident)
        bd_sb = qstuff.tile([2 * QL, BPC], F16)
        nc.gpsimd.memset(bd_sb, 0.0)

        w3_sb = singles.tile([128, KC, D], VDT)
        nc.gpsimd.dma_start(
            out=w3_sb, in_=w3_d.rearrange("p (k d) -> p k d", k=KC))

        # scalar queue: qt+pw packed, then q2 (phase A), then w1 odd chunks
        qtpw_sb = qstuff.tile([128, KC * 2 * QL + KC], F16)
        nc.scalar.dma_start(out=qtpw_sb, in_=qt_d)
        qt_sb = qtpw_sb[:, 0 : KC * 2 * QL].rearrange("p (k q) -> p k q", k=KC)
        pw_sb = qtpw_sb[:, KC * 2 * QL :]
        q2_sb = qstuff.tile([2 * QL, D], F16)
        nc.scalar.dma_start(out=q2_sb, in_=q2_d)
        # qb (512B) before the v8 slabs: it gates the softmax chain, and the
        # one-outstanding-transfer issue backpressure was holding it to ~26us
        qb_sb = qstuff.tile([1, 2 * QL], F32)
        nc.scalar.dma_start(out=qb_sb, in_=qb_d)
        load_v8(nc.scalar, 0)
        load_v8(nc.scalar, 1)

        # sync-queue: small mid-kernel tensors right after vt_s0
        biasc_sb = singles.tile([2, D], F32)
        nc.sync.dma_start(out=biasc_sb, in_=biasc_d)
        vb_sb = qstuff.tile([1, BPC, VL], F32)
        for b in range(BPC):
            nc.sync.dma_start(out=vb_sb[:, b, :], in_=vb_d[b : b + 1, :])
        # aug lhsT rows: [sim; 1; 1] per slab, replicated at partition
        # groups 0-2 and 32-34 so two aug MMs can run packed
        aug_all = qstuff.tile([128, NSLAB, SS], BF16)
        nc.sync.dma_start(out=aug_all[1:3, :, :],
                          in_=aones_d[0:2].rearrange("p (s c) -> p s c", s=NSLAB))
        nc.sync.dma_start(out=aug_all[33:35, :, :],
                          in_=aones_d[2:4].rearrange("p (s c) -> p s c", s=NSLAB))
        # aug rhs: [w2v; bias_hi; bias_lo] per batch, replicated at 0-2/32-34
        aug_rhs = qstuff.tile([128, BPC, D], BF16)
        for b in range(BPC):
            nc.sync.dma_start(out=aug_rhs[0:1, b, :], in_=w2v_d)
            nc.sync.dma_start(out=aug_rhs[32:33, b, :], in_=w2v_d)
        # rest of the sync-queue stream
        load_vt(nc.sync, 1)
        load_vt(nc.sync, 2)
        load_v8(nc.sync, 2)
        load_vt(nc.sync, 3)
        load_v8(nc.sync, 3)

        # ============ Phase A: query side (both batches packed) ==============
        al_ps = psRow.tile([1, 2 * QL], F32, tag="psrow")
        for k in range(KC):
            nc.tensor.matmul(al_ps, pw_sb[:, k : k + 1], qt_sb[:, k, :],
                             start=(k == 0), stop=(k == KC - 1))
        alpha_sb = rows.tile([1, 2 * QL], F32)
        nc.vector.tensor_add(alpha_sb, al_ps, qb_sb)

        alphas_sb = rows.tile([1, 2 * QL], F32, tag="alphas", bufs=1)
        for b in range(BPC):
            seg = slice(b * QL, (b + 1) * QL)
            mx = rows.tile([1, 1], F32)
            nc.vector.reduce_max(mx, alpha_sb[:, seg], axis=AX.X)
            asub = rows.tile([1, QL], F32)
            nc.vector.tensor_scalar_sub(asub, alpha_sb[:, seg], mx)
            aexp = rows.tile([1, QL], F32)
            asum = rows.tile([1, 1], F32)
            nc.scalar.activation(aexp, asub, AF.Exp, accum_out=asum)
            rsum = rows.tile([1, 1], F32)
            nc.vector.reciprocal(rsum, asum)
            nc.vector.tensor_scalar_mul(alphas_sb[:, seg], aexp, rsum)

        sentT = qstuff.tile([128, KC, BPC], F16)
        snsq2 = qstuff.tile([1, BPC], F32)

        def phase_a_rest():
            # alphas^T as block-diagonal [128, 2] fp16 (col b = batch b)
            alT_ps = psRow.tile([2 * QL, 1], F32, tag="psrow")
            nc.tensor.transpose(alT_ps, alphas_sb, ident[:1, :1])
            for b in range(BPC):
                seg = slice(b * QL, (b + 1) * QL)
                nc.vector.tensor_copy(bd_sb[seg, b : b + 1], alT_ps[seg, :])
            # sentence^T chunks: sT[dchunk, b] = Q2^T(chunk) @ bd -> [128,2]/k
            sT_ps = psRow.tile([128, KC * BPC], F32, tag="psrow")
            for k in range(KC):
                nc.tensor.matmul(sT_ps[:, k * BPC : (k + 1) * BPC],
                                 q2_sb[:, k * 128 : (k + 1) * 128], bd_sb,
                                 start=True, stop=True)
            nc.vector.tensor_copy(
                sentT, sT_ps.rearrange("p (k b) -> p k b", k=KC))

            # ||sentence||^2 via self-dot matmuls -> [1, 2], clamped
            sn_ps = psRow.tile([1, BPC], F32, tag="psrow")
            for b in range(BPC):
                for k in range(KC):
                    nc.tensor.matmul(sn_ps[:, b : b + 1],
                                     sentT[:, k, b : b + 1],
                                     sentT[:, k, b : b + 1],
                                     start=(k == 0), stop=(k == KC - 1))
            nc.vector.tensor_scalar_max(snsq2, sn_ps, 1e-16)

        # ================= Phase C: video side (heavy) =======================
        pend_aug = []  # (o_ps pair, slab, b, i) awaiting aug+relu+store

        def flush_aug_pair(entries):
            for h in range(2):
                for j, (o_ps2, slab_j, b_j, i_j) in enumerate(entries):
                    nc.tensor.matmul(
                        o_ps2[h],
                        aug_all[32 * j : 32 * j + 3, slab_j,
                                i_j * 128 : (i_j + 1) * 128],
                        aug_rhs[32 * j : 32 * j + 3, b_j,
                                h * 512 : (h + 1) * 512],
                        start=False, stop=True)
            for o_ps2, slab_j, b_j, i_j in entries:
                out_sb = opool.tile([128, D], F32)
                for h in range(2):
                    nc.vector.tensor_scalar_max(
                        out_sb[:, h * 512 : (h + 1) * 512], o_ps2[h], 0.0)
                r0 = (slab_j % NSS) * SS + i_j * 128
                nc.gpsimd.dma_start(out=out_d[b_j, r0 : r0 + 128, :], in_=out_sb)

        def dot_and_sim(slab):
            # dot row + sim chain for one slab (all at partition 0)
            b, ss = divmod(slab, NSS)
            v8_sb = v8_tiles[slab]
            vnsq = rows.tile([128, 4], F32, tag="vnsqc")
            for s4 in range(4):
                vtrash = trashp.tile([128, D], BF16, tag="vtrash")
                nc.scalar.activation(vtrash, v8_sb[:, s4, :], AF.Square,
                                     accum_out=vnsq[:, s4 : s4 + 1])
            vnsq_bf = rows.tile([128, 4], BF16, tag="vnsqb")
            nc.vector.tensor_copy(vnsq_bf, vnsq)

            dps = psDot.tile([1, SS], F32, tag="dot")
            for k in range(KC):
                nc.tensor.matmul(dps, sentT[:, k, b : b + 1],
                                 vt_tiles[slab][:, k, :],
                                 start=(k == 0), stop=(k == KC - 1))
            # norm row: one [128,4]->[4,128] bf16 transpose, then a small
            # DMA gathers the psum rows into a [1,512] sbuf row
            vnr_ps = psRow.tile([4, 128], BF16, tag="psrow")
            nc.tensor.transpose(vnr_ps, vnsq_bf, identB)
            vnr_cp = rows.tile([4, 128], BF16, tag="vnrcp")
            nc.vector.tensor_copy(vnr_cp, vnr_ps)
            vnr_row = rows.tile([1, SS], BF16, tag="vnrrow")
            nc.gpsimd.dma_start(out=vnr_row, in_=vnr_cp)

            # sim = dot * rsqrt(max(vnsq,eps)*snsq) + log(video_mask)
            t1 = rows.tile([1, SS], F32, tag="t1")
            nc.vector.tensor_scalar(t1, vnr_row, 1e-16, snsq2[:, b : b + 1],
                                    op0=ALU.max, op1=ALU.mult)
            t3 = rows.tile([1, SS], F32, tag="t3")
            nc.scalar.activation(t3, t1, AF.Abs_reciprocal_sqrt)
            t4 = rows.tile([1, SS], F32, tag="t4")
            nc.vector.tensor_mul(t4, dps, t3)
            sim_row = rows.tile([1, SS], BF16, tag="simrow")
            nc.vector.tensor_add(sim_row, t4,
                                 vb_sb[:, b, ss * SS : (ss + 1) * SS])
            nc.gpsimd.dma_start(out=aug_all[0:1, slab, :], in_=sim_row)
            nc.gpsimd.dma_start(out=aug_all[32:33, slab, :], in_=sim_row)

        def bias_rows():
            # bias row [2, D] = sentence @ W3 + biasc; h halves col-packed
            b_ps = psRow.tile([128, 512], F32, tag="psrow")
            for k in range(KC):
                nc.tensor.matmul(b_ps[0:2, :], sentT[:, k, :],
                                 w3_sb[:, k, 0:512],
                                 start=(k == 0), stop=(k == KC - 1))
                nc.tensor.matmul(b_ps[32:34, :], sentT[:, k, :],
                                 w3_sb[:, k, 512:1024],
                                 start=(k == 0), stop=(k == KC - 1))
            bias_f = rows.tile([2, D], F32, tag="biasf", bufs=1)
            nc.vector.tensor_add(bias_f[:, 0:512], b_ps[0:2, :],
                                 biasc_sb[:, 0:512])
            # partition-32 psum half lands via a bounce + DMA hop
            bh_cp = rows.tile([34, 512], F32, tag="bhcp", bufs=1)
            nc.vector.tensor_copy(bh_cp[32:34, :], b_ps[32:34, :])
            b_hi = rows.tile([2, 512], F32, tag="bhi", bufs=1)
            nc.gpsimd.dma_start(out=b_hi, in_=bh_cp[32:34, :])
            nc.vector.tensor_add(bias_f[:, 512:1024], b_hi, biasc_sb[:, 512:1024])
            bias_hi = rows.tile([2, D], BF16, tag="biashi", bufs=1)
            nc.vector.tensor_copy(bias_hi, bias_f)
            bias_lo = rows.tile([2, D], BF16, tag="biaslo", bufs=1)
            nc.vector.tensor_sub(bias_lo, bias_f, bias_hi)
            for b2 in range(BPC):
                for p in (1, 33):
                    nc.gpsimd.dma_start(out=aug_rhs[p : p + 1, b2, :],
                                        in_=bias_hi[b2 : b2 + 1, :])
                    nc.gpsimd.dma_start(out=aug_rhs[p + 1 : p + 2, b2, :],
                                        in_=bias_lo[b2 : b2 + 1, :])

        # slab 0 is decoupled: its main partials leave PSUM immediately
        # (banks recycle without waiting on the sim/bias chain); the rank-3
        # aug is applied later into fresh banks + DVE add/relu.
        decoup = []

        def late_assemble(entry):
            dec_sb, slab_j, b_j, i_j = entry
            out_sb = opool.tile([128, D], F32)
            for h in range(2):
                a_ps = psMain.tile([128, 512], F32, tag="o_ps",
                                   name=f"late_{i_j}_{h}")
                nc.tensor.matmul(
                    a_ps, aug_all[0:3, slab_j, i_j * 128 : (i_j + 1) * 128],
                    aug_rhs[0:3, b_j, h * 512 : (h + 1) * 512],
                    start=True, stop=True)
                nc.vector.tensor_add(out_sb[:, h * 512 : (h + 1) * 512],
                                     dec_sb[:, h * 512 : (h + 1) * 512], a_ps)
                nc.vector.tensor_scalar_max(
                    out_sb[:, h * 512 : (h + 1) * 512],
                    out_sb[:, h * 512 : (h + 1) * 512], 0.0)
            r0 = (slab_j % NSS) * SS + i_j * 128
            nc.gpsimd.dma_start(out=out_d[b_j, r0 : r0 + 128, :], in_=out_sb)

        bias_done = False
        for slab in range(NSLAB):
            b, ss = divmod(slab, NSS)
            vt_sb = vt_tiles[slab]
            thresh = 2 if slab == NSLAB - 1 else 3
            for i in range(4):
                if slab == 1 and i < 2:
                    late_assemble(decoup.pop(0))
                    late_assemble(decoup.pop(0))
                if len(pend_aug) >= thresh:
                    flush_aug_pair([pend_aug.pop(0), pend_aug.pop(0)])

                # phase-A rest issues BEFORE t1's mains/copies so its DVE ops
                # queue ahead of the decoupling copies (they were delaying the
                # block-diagonal build and stalling the sentence MMs ~3us)
                if slab == 0 and i == 1:
                    phase_a_rest()

                o_ps2 = [psMain.tile([128, 512], F32, tag="o_ps",
                                     name=f"o_ps_{slab}_{i}_{h}")
                         for h in range(2)]
                # 8 consecutive MMs per PSUM bank (bank alternation per
                # instruction triggers the PE depth-cycling penalty)
                for h in range(2):
                    for k in range(KC):
                        nc.tensor.matmul(
                            o_ps2[h], vt_sb[:, k, i * 128 : (i + 1) * 128],
                            w1_sb[:, k, h * 512 : (h + 1) * 512],
                            start=(k == 0),
                            stop=(slab == 0 and k == KC - 1))
                if slab == 0:
                    dec_sb = decpool.tile([128, D], F32, tag="dec",
                                          name=f"dec_{i}")
                    for h in range(2):
                        nc.vector.tensor_copy(
                            dec_sb[:, h * 512 : (h + 1) * 512], o_ps2[h])
                    decoup.append((dec_sb, slab, b, i))
                else:
                    pend_aug.append((o_ps2, slab, b, i))

                if i == 1:
                    dot_and_sim(slab)
                if i == 2 and not bias_done:
                    bias_rows()
                    bias_done = True

        while pend_aug:
            take = pend_aug[:2]
            pend_aug = pend_aug[2:]
            flush_aug_pair(take)

    nc.compile()
    return nc


_NC = None
_LAST_RESULTS = None


def _get_program():
    global _NC
    if _NC is None:
        _NC = _build_program()
    return _NC


def kernel(video_features, query_features, video_mask, query_mask,
           sim_w, cor_v_w, cor_q_w, pool_w, mixer_w, mixer_b):
    video_features = np.asarray(video_features, dtype=np.float32)
    query_features = np.asarray(query_features, dtype=np.float32)
    video_mask = np.asarray(video_mask, dtype=np.float32)
    query_mask = np.asarray(query_mask, dtype=np.float32)
    sim_w = np.asarray(sim_w, dtype=np.float32)
    cor_v_w = np.asarray(cor_v_w, dtype=np.float32)
    cor_q_w = np.asarray(cor_q_w, dtype=np.float32)
    pool_w = np.asarray(pool_w, dtype=np.float32)
    mixer_w = np.asarray(mixer_w, dtype=np.float32)
    mixer_b = np.asarray(mixer_b, dtype=np.float32)

    # host-side folds of the weight-only algebra (O(d^2), negligible)
    W1 = np.ascontiguousarray(mixer_w[0:D]).astype(np.float16)
    W2 = mixer_w[D : 2 * D]
    W3 = np.ascontiguousarray(mixer_w[2 * D : 3 * D]).astype(np.float16)
    W4 = mixer_w[3 * D : 4 * D]
    w2v = (sim_w[:, 0] @ W2.astype(np.float32)).astype(ml_dtypes.bfloat16)[None, :]
    cor_vec = (cor_v_w[0] * cor_q_w[0, 0]).astype(np.float32)
    biasc = np.broadcast_to((cor_vec @ W4 + mixer_b).astype(np.float32)[None, :],
                            (2, D)).copy()
    qbias = ((1.0 - query_mask) * NEG_INF).astype(np.float32)
    vbias = np.log(video_mask + 1e-45).astype(np.float32)
    pw2 = np.ascontiguousarray(pool_w[:, 0].reshape(KC, 128).T).astype(np.float16)
    aones = np.ones((4, NSLAB * SS), dtype=ml_dtypes.bfloat16)

    v16 = video_features.astype(np.float16)
    # vt: [BS*NSS, 128, KC*SS] with vt[slab, p, k*SS+c] = V[b, ss*SS+c, k*128+p]
    vt_full = np.ascontiguousarray(
        v16.transpose(0, 2, 1)                 # [BS, D, VL]
        .reshape(BS, KC, 128, NSS, SS)
        .transpose(0, 3, 2, 1, 4)              # [BS, NSS, 128, KC, SS]
        .reshape(BS * NSS, 128, KC * SS))
    # v8: [BS*NSS, 128, 4*D] with v8[slab, p, s4*D+d] = V[b, ss*SS+s4*128+p, d]
    v8_full = np.ascontiguousarray(
        video_features.astype(ml_dtypes.float8_e4m3fn)
        .reshape(BS, NSS, 4, 128, D)
        .transpose(0, 1, 3, 2, 4)              # [BS, NSS, 128, 4, D]
        .reshape(BS * NSS, 128, 4 * D))
    q16 = query_features.astype(np.float16)
    W1s = np.ascontiguousarray(                # [128, KC*D]
        W1.reshape(KC, 128, D).transpose(1, 0, 2).reshape(128, KC * D))
    W3s = np.ascontiguousarray(
        W3.reshape(KC, 128, D).transpose(1, 0, 2).reshape(128, KC * D))

    nc = _get_program()
    in_maps = []
    for c in range(NCORES):
        sl = slice(c * BPC, (c + 1) * BPC)
        qc = q16[sl]                                             # [2, QL, D]
        qt = np.ascontiguousarray(np.concatenate([  # [128, KC*2QL + KC]
            qc.transpose(2, 0, 1).reshape(KC, 128, 2 * QL)
            .transpose(1, 0, 2).reshape(128, KC * 2 * QL),
            pw2], axis=1))
        q2 = np.ascontiguousarray(qc.reshape(2 * QL, D))
        slsl = slice(c * NSLAB, (c + 1) * NSLAB)
        in_maps.append({
            "vt": vt_full[slsl],
            "v8": v8_full[slsl],
            "qt": qt,
            "q2": q2,
            "qb": np.ascontiguousarray(qbias[sl].reshape(1, 2 * QL)),
            "vb": np.ascontiguousarray(vbias[sl]),
            "w1": W1s,
            "w3": W3s,
            "w2v": w2v,
            "biasc": biasc,
            "aones": aones,
        })
    res = run_bass_kernel_spmd(nc, in_maps, core_ids=list(range(NCORES)))
    global _LAST_RESULTS
    _LAST_RESULTS = res
    out = np.concatenate([res.results[c]["out"] for c in range(NCORES)], axis=0)
    return out.astype(np.float32, copy=False)
